# revision 9
# baseline (speedup 1.0000x reference)
"""Causal multi-head attention for TRN2, sharded across 8 NeuronCores.

Problem: x[4,2048,1024] -> 16-head causal self-attention (head_dim 64) with
QKV + output projections, fp32.

Sharding: core c -> batch b = c // 2, head-group g = c % 2 (heads g*8..g*8+7).
Per core: Q/K/V projections use the 512 weight columns of its head-group
(column-parallel); attention runs over its 8 heads; the output projection
uses the matching 512 rows of wo (row-parallel), so each core emits a
partial [2048,1024] output and the host sums the two partials per batch.
bo is added on the g==0 cores only (g==1 cores receive zeros).

Device design (per core; S=2048, D=1024, HD=64; matmul operands bf16, all
accumulation fp32 in PSUM):
  - Heads processed as PAIRS: head 2t in SBUF partitions 0:64, head 2t+1
    in 64:128 (QT/KT/AT tiles [128, 4, S]). Off-diagonal score matmuls are
    64-row TILED (tile T0/T8 via base_partition 0/64): both heads' scores
    run CONCURRENTLY on the PE (measured 2.0x). Mode switches (64<->128)
    cost ~106ns, so tiled scores are emitted in 2-round blocks
    ([S,S] 64-mode | [filler+AV...] 128-mode) and the 4 short DIAGONAL
    rounds per pair-chunk run un-tiled (128-contraction, zero-padded via a
    small per-chunk KDz tile) inside the 128-mode region - for short
    streams the switch costs more than tiling saves.
  - Scores land in [128, 2, 512] PSUM tiles (2 adjacent banks); exp reads
    both heads' banks in ONE batched scalar-engine instruction (halves the
    per-instruction PSUM-latency overhead; ACT is the chunk-3
    co-bottleneck).
  - An all-ones column per head's V block accumulates softmax denominators
    in psum row 64 (AV cost is per-streamed-column, so output-partition
    padding is free). Causal: per k-block only the valid q-range is
    computed; the diagonal 128x128 block is masked by a DVE multiply with
    an upper-triangular tile.
  - Normalization per head: scalar-engine Reciprocal directly from PSUM
    (chunk 3 pairs 0-2 use a DVE copy + reciprocal_approx_fast instead,
    since ACT is saturated by exp there), gpsimd partition-broadcast, DVE
    multiply.
  - Startup: big inputs stream as ~1MB DMAs in consumption order (wq,
    xT in 2-d-slice chunks, wk, ...); wv/wo DMAs are deferred via marker
    dependencies so they don't steal HBM bandwidth from the critical path.
    The chunk-0 projections are emitted d-major over 4 single-bank PSUM
    tiles so the PE tracks DMA arrivals; dense dummy matmuls at t=0 trip
    the PE HAM clock-gate to full rate before real work lands. Zero/one
    fills use on-chip memset (no DMA).
  - Pipeline: chunk j's attention interleaves chunk j+1's projections as
    PE filler; ALL out-projections (chunks 0..2) fill chunk 3 (whose
    attention is ACT-exp-bound); chunk 3's own out-projection is the tail,
    bridged by keep-warm dummies.
"""

import os
from contextlib import ExitStack

import numpy as np

import concourse.bacc as bacc
import concourse.mybir as mybir
import concourse.tile as tile
from concourse.bass_utils import run_bass_kernel_spmd
from concourse.masks import make_upper_triangular

F32 = mybir.dt.float32
BF16 = mybir.dt.bfloat16
AF = mybir.ActivationFunctionType
ALU = mybir.AluOpType

B = 4
S = 2048
D = 1024
HD = 64
HG = 8  # heads per core
QC = HG * HD  # 512 local q/k/v columns
N_CORES = 8

_NC_CACHE = {}
LAST_RESULT = None  # BassKernelResults of the most recent kernel() call


def _build_nc(s: int = S, num_devices: int = N_CORES):
    P = 128
    NQ = s // 512
    NS = s // P
    ND = D // P
    NT = QC // P  # 4 head pairs
    VW = HD + 1  # 65: per-head V block width (64 cols + ones col)
    VPAD = 7 * VW + P  # 583: last head's lhsT slice must fit

    nc = bacc.Bacc("TRN2", target_bir_lowering=False, debug=False, num_devices=num_devices)

    xT_d = nc.dram_tensor("xT", [D, s], BF16, kind="ExternalInput").ap()
    wq_d = nc.dram_tensor("wq", [D, QC], BF16, kind="ExternalInput").ap()
    wk_d = nc.dram_tensor("wk", [D, QC], BF16, kind="ExternalInput").ap()
    wv_d = nc.dram_tensor("wv", [D, QC], BF16, kind="ExternalInput").ap()
    wo_d = nc.dram_tensor("wo", [QC, D], BF16, kind="ExternalInput").ap()
    bq_d = nc.dram_tensor("bq", [QC], F32, kind="ExternalInput").ap()
    bk_d = nc.dram_tensor("bk", [QC], F32, kind="ExternalInput").ap()
    bv_d = nc.dram_tensor("bv", [QC], F32, kind="ExternalInput").ap()
    bo_d = nc.dram_tensor("bo", [D], F32, kind="ExternalInput").ap()
    out_d = nc.dram_tensor("out", [s, D], F32, kind="ExternalOutput").ap()

    with tile.TileContext(nc) as tc:
        with ExitStack() as ctx:
            consts = ctx.enter_context(tc.tile_pool(name="consts", bufs=1))
            persist = ctx.enter_context(tc.tile_pool(name="persist", bufs=1))
            e_pool = ctx.enter_context(tc.tile_pool(name="epool", bufs=4))
            n_pool = ctx.enter_context(tc.tile_pool(name="npool", bufs=4))
            b_pool = ctx.enter_context(tc.tile_pool(name="bpool", bufs=4))
            o_pool = ctx.enter_context(tc.tile_pool(name="opool", bufs=3))
            proj_psum = ctx.enter_context(tc.tile_pool(name="proj_ps", bufs=2, space="PSUM"))
            s_psum = ctx.enter_context(tc.tile_pool(name="s_ps", bufs=2, space="PSUM"))
            a_psum = ctx.enter_context(tc.tile_pool(name="a_ps", bufs=2, space="PSUM"))

            # ---- tri mask first (no DMA deps) so warmup can start at t~0 ----
            tri = consts.tile([P, P], F32)
            make_upper_triangular(nc, tri[:], val=1.0, diag=True)
            tri_b = consts.tile([P, P], BF16)
            nc.any.tensor_copy(tri_b[:], tri[:])

            def dummy(n=1):
                """Keep-warm matmuls (tri x tri into a rotating proj bank)."""
                kw = proj_psum.tile([P, 512], F32, tag="pp", name="kw")
                for _ in range(n):
                    nc.tensor.matmul(
                        kw[:, 0:P], lhsT=tri_b[:], rhs=tri_b[:], start=True, stop=True
                    )

            dummy(32)  # ~3.4us of PE activity trips HAM to 8/8 before real work

            # ---- small consts ----
            bqc = consts.tile([P, NT], F32)
            bkc = consts.tile([P, NT], F32)
            nc.sync.dma_start(bqc[:], bq_d.rearrange("(t p) -> p t", p=P))
            nc.sync.dma_start(bkc[:], bk_d.rearrange("(t p) -> p t", p=P))
            bv1 = consts.tile([1, QC], F32)
            bo1 = consts.tile([1, D], F32)
            nc.sync.dma_start(bv1[:], bv_d[None, :])
            nc.sync.dma_start(bo1[:], bo_d[None, :])
            bvb = consts.tile([P, QC], F32)
            bob = consts.tile([P, D], F32)
            nc.gpsimd.partition_broadcast(bvb[:], bv1[0:1, :])
            nc.gpsimd.partition_broadcast(bob[:], bo1[0:1, :])

            # ---- persistent SBUF tensors ----
            QT = persist.tile([P, NT, s], BF16)
            KT = persist.tile([P, NT, s], BF16)
            # per-chunk diag K, zero-padded; double-buffered by chunk parity
            # (chunk j+1's K filler evac must not collide with chunk j's reads)
            KDz = persist.tile([P, HG, 2, 512], BF16)
            V = persist.tile([P, NS, VPAD + 1], BF16)
            AT = persist.tile([P, NT, s], BF16)
            xT = persist.tile([P, ND, s], BF16)
            wq_sb = persist.tile([P, ND, QC], BF16)
            wk_sb = persist.tile([P, ND, QC], BF16)
            wv_sb = persist.tile([P, ND, QC], BF16)
            wo_sb = persist.tile([P, NT, D], BF16)

            # V pad/ones + KDz constant zero halves, on-chip
            nc.any.memset(V[:, :, 7 * VW + HD + 1 :], 0.0)
            nc.any.memset(
                V[:, :, 0 : HG * VW].rearrange("p s (h c) -> p s h c", c=VW)[:, :, :, HD : HD + 1],
                1.0,
            )
            nc.any.memset(
                KDz[64:128].rearrange("p (t two) pr c -> p t two pr c", two=2)[:, :, 0], 0.0
            )
            nc.any.memset(
                KDz[0:64].rearrange("p (t two) pr c -> p t two pr c", two=2)[:, :, 1], 0.0
            )

            # ---- big input DMAs, ~1MB each, in consumption order ----
            nc.sync.dma_start(wq_sb[:], wq_d.rearrange("(nd p) c -> p nd c", p=P))
            for dd in range(0, ND, 2):
                nc.sync.dma_start(
                    xT[:, dd : dd + 2, :],
                    xT_d[dd * P : (dd + 2) * P, :].rearrange("(a p) s -> p a s", p=P),
                )
                if dd == 0:
                    nc.sync.dma_start(wk_sb[:], wk_d.rearrange("(nd p) c -> p nd c", p=P))
            # wv/wo deferred via marker deps: don't steal HBM bandwidth from
            # the critical path above. Marker write -> WAW forces DMA order.
            nc.vector.tensor_copy(wv_sb[0:1, 0, 0:2], xT[0:1, 4, 0:2])
            nc.sync.dma_start(wv_sb[:, 0:4, :], wv_d[0 : 4 * P, :].rearrange("(a p) c -> p a c", p=P))
            nc.sync.dma_start(wv_sb[:, 4:8, :], wv_d[4 * P : 8 * P, :].rearrange("(a p) c -> p a c", p=P))
            nc.vector.tensor_copy(wo_sb[0:1, 0, 0:2], xT[0:1, 6, 0:2])
            nc.sync.dma_start(wo_sb[:], wo_d.rearrange("(nt p) c -> p nt c", p=P))

            # ---- evacuation helpers ----
            def evac_q(ps, t, j):
                js = slice(j * 512, (j + 1) * 512)
                nc.vector.tensor_scalar_add(QT[:, t, js], ps[:], bqc[:, t : t + 1])

            def evac_k(ps, t, j):
                js = slice(j * 512, (j + 1) * 512)
                nc.vector.tensor_scalar_add(KT[:, t, js], ps[:], bkc[:, t : t + 1])
                # diag copy for chunk j (zero halves are persistent)
                nc.vector.tensor_scalar_add(
                    KDz[0:64, 2 * t, j % 2, :], ps[0:64, :], bkc[0:64, t : t + 1]
                )
                nc.vector.tensor_scalar_add(
                    KDz[64:128, 2 * t + 1, j % 2, :], ps[64:128, :], bkc[64:128, t : t + 1]
                )

            def evac_v(ps, st):
                dst = V[:, st, 0 : HG * VW].rearrange("p (h c) -> p h c", c=VW)[:, :, 0:HD]
                src = ps.rearrange("p (h c) -> p h c", c=HD)
                bsrc = bvb.rearrange("p (h c) -> p h c", c=HD)
                nc.vector.tensor_tensor(dst, src, bsrc, ALU.add)

            # ---- startup chunk-0 projections, d-major over 4 banks ----
            def wave4(make_mm, evacs):
                slots = [
                    proj_psum.tile([P, 512], F32, tag="pp", name="wv0"),
                    proj_psum.tile([P, 512], F32, tag="pp", name="wv1"),
                    a_psum.tile([P, 512], F32, tag="a", name="wv2"),
                    a_psum.tile([P, 512], F32, tag="a", name="wv3"),
                ]
                for d in range(ND):
                    for i in range(4):
                        make_mm(slots[i], i, d)
                for i in range(4):
                    evacs(slots[i], i)

            wave4(
                lambda ps, t, d: nc.tensor.matmul(
                    ps[:], lhsT=wq_sb[:, d, t * P : (t + 1) * P], rhs=xT[:, d, 0:512],
                    start=(d == 0), stop=(d == ND - 1),
                ),
                lambda ps, t: evac_q(ps, t, 0),
            )
            wave4(
                lambda ps, t, d: nc.tensor.matmul(
                    ps[:], lhsT=wk_sb[:, d, t * P : (t + 1) * P], rhs=xT[:, d, 0:512],
                    start=(d == 0), stop=(d == ND - 1),
                ),
                lambda ps, t: evac_k(ps, t, 0),
            )
            wave4(
                lambda ps, st, d: nc.tensor.matmul(
                    ps[:], lhsT=xT[:, d, st * P : (st + 1) * P], rhs=wv_sb[:, d, :],
                    start=(d == 0), stop=(d == ND - 1),
                ),
                lambda ps, st: evac_v(ps, st),
            )

            # ---- filler units ----
            def proj_group(j, g):
                """One psum-group of the j-chunk projections; g in 0..11."""
                js = slice(j * 512, (j + 1) * 512)
                kind, t = divmod(g, NT)
                ps = proj_psum.tile([P, 512], F32, tag="pp", name="pp")
                if kind == 0:  # Q
                    for d in range(ND):
                        nc.tensor.matmul(
                            ps[:],
                            lhsT=wq_sb[:, d, t * P : (t + 1) * P],
                            rhs=xT[:, d, js],
                            start=(d == 0),
                            stop=(d == ND - 1),
                        )
                    evac_q(ps, t, j)
                elif kind == 1:  # K
                    for d in range(ND):
                        nc.tensor.matmul(
                            ps[:],
                            lhsT=wk_sb[:, d, t * P : (t + 1) * P],
                            rhs=xT[:, d, js],
                            start=(d == 0),
                            stop=(d == ND - 1),
                        )
                    evac_k(ps, t, j)
                else:  # V s-tile 4j+t
                    st = 4 * j + t
                    for d in range(ND):
                        nc.tensor.matmul(
                            ps[:],
                            lhsT=xT[:, d, st * P : (st + 1) * P],
                            rhs=wv_sb[:, d, :],
                            start=(d == 0),
                            stop=(d == ND - 1),
                        )
                    evac_v(ps, st)

            def out_proj_group(j, g):
                st = 4 * j + g // 2
                oc = g % 2
                o_ps = proj_psum.tile([P, 512], F32, tag="pp", name="o_ps")
                for t2 in range(NT):
                    nc.tensor.matmul(
                        o_ps[:],
                        lhsT=AT[:, t2, st * P : (st + 1) * P],
                        rhs=wo_sb[:, t2, oc * 512 : (oc + 1) * 512],
                        start=(t2 == 0),
                        stop=(t2 == NT - 1),
                    )
                ot = o_pool.tile([P, 512], F32, name="ot")
                nc.vector.tensor_tensor(
                    ot[:], o_ps[:], bob[:, oc * 512 : (oc + 1) * 512], ALU.add
                )
                nc.sync.dma_start(
                    out_d[st * P : (st + 1) * P, oc * 512 : (oc + 1) * 512], ot[:]
                )

            # ---- attention pair-chunk ----
            def attn_pair(j, t, filler, f_lo, f_hi, act_recip):
                """Heads (2t, 2t+1) on q-chunk j. filler[f_lo:f_hi] emitted in
                128-mode regions. Slots: one per tiled 2-round block + 2 in
                the diagonal region."""
                nkb = 4 * j + 4
                ntb = 2 * j  # tiled (off-diagonal) 2-round blocks
                nslot = ntb + 2
                A0 = a_psum.tile([P, 512], F32, tag="a", name="A0")
                A1 = a_psum.tile([P, 512], F32, tag="a", name="A1")
                jq = j * 512
                nfill = f_hi - f_lo
                slot = 0

                def fill_slot():
                    nonlocal slot
                    k0 = f_lo + (nfill * slot) // nslot
                    k1 = f_lo + (nfill * (slot + 1)) // nslot
                    for f in filler[k0:k1]:
                        f()
                    slot += 1

                def exp_round(r, y0, S2):
                    E2 = e_pool.tile([P, 2, 512], BF16, tag="e", name="E2")
                    nc.scalar.activation(
                        E2[:, :, y0:], S2[:, :, y0:], AF.Exp, scale=0.125
                    )
                    if r >= 4 * j:  # diagonal 128-block: causal mask
                        for i in range(2):
                            nc.vector.tensor_tensor(
                                E2[:, i, y0 : y0 + P],
                                E2[:, i, y0 : y0 + P],
                                tri_b[:],
                                ALU.mult,
                            )
                    return E2

                def av_round(r, y0, E2):
                    nc.tensor.matmul(
                        A0[:, y0:],
                        lhsT=V[:, r, (2 * t) * VW : (2 * t) * VW + P],
                        rhs=E2[:, 0, y0:],
                        start=(r == 0),
                        stop=(r == nkb - 1),
                    )
                    nc.tensor.matmul(
                        A1[:, y0:],
                        lhsT=V[:, r, (2 * t + 1) * VW : (2 * t + 1) * VW + P],
                        rhs=E2[:, 1, y0:],
                        start=(r == 0),
                        stop=(r == nkb - 1),
                    )

                # off-diagonal: 64-row-tiled score pairs, 2 rounds per block
                for blk in range(ntb):
                    rr = (2 * blk, 2 * blk + 1)
                    Ss = []
                    for r in rr:
                        S2 = s_psum.tile([P, 2, 512], F32, tag="s", name="S2")
                        nc.tensor.matmul(
                            S2[:, 0, :],
                            lhsT=KT[0:64, t, r * P : (r + 1) * P],
                            rhs=QT[0:64, t, jq : jq + 512],
                            start=True,
                            stop=True,
                        )
                        nc.tensor.matmul(
                            S2[:, 1, :],
                            lhsT=KT[64:128, t, r * P : (r + 1) * P],
                            rhs=QT[64:128, t, jq : jq + 512],
                            start=True,
                            stop=True,
                        )
                        Ss.append(S2)
                    Es = [(r, 0, exp_round(r, 0, S2)) for r, S2 in zip(rr, Ss)]
                    fill_slot()
                    for r, y0, E2 in Es:
                        av_round(r, y0, E2)

                # diagonal region: 4 un-tiled (128-contraction) rounds
                for half in range(2):
                    Es = []
                    for i in range(2):
                        r = 4 * j + 2 * half + i
                        y0 = P * (r - 4 * j)
                        S2 = s_psum.tile([P, 2, 512], F32, tag="s", name="S2d")
                        for hh in range(2):
                            nc.tensor.matmul(
                                S2[:, hh, y0:],
                                lhsT=KDz[:, 2 * t + hh, j % 2, y0 : y0 + P],
                                rhs=QT[:, t, jq + y0 : jq + 512],
                                start=True,
                                stop=True,
                            )
                        Es.append((r, y0, exp_round(r, y0, S2)))
                    fill_slot()
                    for r, y0, E2 in Es:
                        av_round(r, y0, E2)

                # softmax normalization for both heads
                for i, A in enumerate((A0, A1)):
                    rec = n_pool.tile([1, 512], F32, tag="rec", name="rec")
                    sums = n_pool.tile([1, 512], F32, tag="sums", name="sums")
                    if act_recip:
                        # ACT pulls the PSUM row (frees the DVE queue position)
                        nc.scalar.activation(sums[:], A[HD : HD + 1, :], AF.Copy)
                    else:
                        nc.vector.tensor_copy(sums[:], A[HD : HD + 1, :])
                    nc.vector.reciprocal_approx_fast(rec[:], sums[:])
                    bc = b_pool.tile([HD, 512], F32, name="bc")
                    nc.gpsimd.partition_broadcast(bc[:], rec[0:1, :])
                    nc.vector.tensor_tensor(
                        AT[64 * i : 64 * i + HD, t, jq : jq + 512],
                        A[0:HD, :],
                        bc[:],
                        ALU.mult,
                    )

            # ---- main pipeline ----
            for j in range(NQ):
                if j < NQ - 1:
                    filler = [
                        (lambda jj=j + 1, g=g: proj_group(jj, g)) for g in range(12)
                    ]
                else:
                    filler = [
                        (lambda jj=jo, g=g: out_proj_group(jj, g))
                        for jo in range(NQ - 1)
                        for g in range(8)
                    ]
                nf = len(filler)
                for t in range(NT):
                    attn_pair(
                        j, t, filler, (nf * t) // NT, (nf * (t + 1)) // NT,
                        act_recip=(j < NQ - 1 or t == NT - 1),
                    )

            # tail: bridge the last normalization, then chunk-3 out-proj
            dummy(16)
            for g in range(8):
                out_proj_group(NQ - 1, g)

    nc.compile()

    return nc


def _get_nc():
    if "nc" not in _NC_CACHE:
        _NC_CACHE["nc"] = _build_nc()
    return _NC_CACHE["nc"]


def make_in_maps(x, wq, bq, wk, bk, wv, bv, wo, bo, n_cores=N_CORES):
    import ml_dtypes

    bf = ml_dtypes.bfloat16
    x = np.asarray(x, np.float32).astype(bf)
    wq, wk, wv, wo = (np.asarray(a, np.float32).astype(bf) for a in (wq, wk, wv, wo))
    bq, bk, bv, bo = (np.asarray(a, np.float32) for a in (bq, bk, bv, bo))
    in_maps = []
    for c in range(n_cores):
        b, g = c // 2, c % 2
        cs = slice(g * QC, (g + 1) * QC)
        in_maps.append(
            {
                "xT": np.ascontiguousarray(x[b].T),
                "wq": np.ascontiguousarray(wq[:, cs]),
                "wk": np.ascontiguousarray(wk[:, cs]),
                "wv": np.ascontiguousarray(wv[:, cs]),
                "wo": np.ascontiguousarray(wo[cs, :]),
                "bq": np.ascontiguousarray(bq[cs]),
                "bk": np.ascontiguousarray(bk[cs]),
                "bv": np.ascontiguousarray(bv[cs]),
                "bo": bo if g == 0 else np.zeros_like(bo),
            }
        )
    return in_maps


def kernel(x, wq, bq, wk, bk, wv, bv, wo, bo):
    global LAST_RESULT
    in_maps = make_in_maps(x, wq, bq, wk, bk, wv, bv, wo, bo)
    nc = _get_nc()
    trace = os.environ.get("MHA_TRACE", "0") == "1"
    res = run_bass_kernel_spmd(nc, in_maps, core_ids=list(range(N_CORES)), trace=trace)
    LAST_RESULT = res

    out = np.empty((B, S, D), np.float32)
    for b in range(B):
        out[b] = res.results[2 * b]["out"] + res.results[2 * b + 1]["out"]
    return out
